# revision 7
# baseline (speedup 1.0000x reference)
"""Trainium2 Bass kernel for nn_Attention_9689446220043.

Computation (per batch b):
    left  = x @ W1            [A, R]
    right = W2 @ x^T          [R, A]
    S     = left @ right      [A, A]
    P     = softmax(S / sqrt(512), axis=-1)
    out   = P @ x             [A, D]

Strategy (8 NeuronCores, data-parallel over batch B=16 -> 2 batches/core):
  - Work in the *transposed* score layout S^T[c, a] so the PV matmul
    (out = P @ x) needs no transpose of P: out[a-tile] = P^T[:, a-slice].T @ x.
  - softmax without max-subtraction (scores/sqrt(512) is within [-1.5, 1.5]
    for randn inputs, exp is safe) and with *deferred* normalization:
    out = (expS^T).T @ x, then divide rows by sumexp.
  - sumexp folded into the PV loop as N=1 matmuls reusing the PV weights
    (duplicate LDWEIGHTS elided by a custom pass), issued *before* the
    N=512 matmul of each step so the next LDWEIGHTS prefetches under the
    long matmul.
  - All matmul operands bf16 (PE streams 1 col/cycle; fp32 would be 4x),
    accumulation fp32 in PSUM. Projection weights zero-padded to M=128
    columns so fast-weight-load kicks in.
  - Head: warmup matmuls feed off the wcat DMA (sync queue starts ~3us
    before the compute engines), identity comes from DRAM, first x tiles
    ride the sync queue, and score steps are (a-slice, ct-pair) granular
    so scoring starts as soon as projection chunk 0 lands.
  - Tail: normalization split across Vector/Scalar halves with split
    output DMAs.
"""

import sys

if "/opt/trn_rl_repo" not in sys.path:
    sys.path.insert(0, "/opt/trn_rl_repo")

import ml_dtypes
import numpy as np

import concourse.bass as bass
import concourse.tile as tile
from concourse import mybir
from concourse.bass_utils import run_bass_kernel_spmd
from concourse.vector_clock import ScopedClock

# Problem shape (hardcoded per contract).
B, A, D, R = 16, 2048, 512, 10
NCORES = 8
PB = B // NCORES  # batches per core
P = 128
AT = A // P  # a-tiles (16)
CT = A // P  # c-tiles (16)
DC = D // P  # d-chunks (4)
SCALE = float(1.0 / np.sqrt(512.0))

F32 = mybir.dt.float32
DT = mybir.dt.bfloat16
NP_DT = ml_dtypes.bfloat16

WARMUP_MMS = 16


class PatchedTileContext(tile.TileContext):
    """Two fixes for this container's walrus build / perf:

    1. walrus rejects instructions carrying more than one semaphore
       sync-wait ("Too many sync wait commands"), and rejects ge-mode waits
       on InstDrain entirely. Hoist excess waits onto standalone
       EventSemaphore (wait) instructions emitted just before the owning
       instruction on the same engine.

    2. Tile splits every matmul into LDWEIGHTS+MATMUL and never dedups;
       walrus ldw-opt is disabled in this toolchain. Drop an LDWEIGHTS that
       reloads exactly the weights already in the PE array (sync-free ones
       only), so back-to-back matmuls sharing lhsT pay one weight load.
    """

    _wsplit_counter = 0

    def __init__(self, *args, **kwargs):
        super().__init__(*args, **kwargs)
        self._last_pe_weights = None
        self.n_ldw_dropped = 0

    def _split_excess_waits(self, inst, original_block):
        si = inst.sync_info
        if si is None:
            return
        waits = list(si.on_wait)
        if isinstance(inst, (mybir.InstDrain, mybir.InstNoOp)):
            keep = [w for w in waits if w.wait_mode == "sem-eq-imm"][:1]
        else:
            keep = waits[-1:]
        hoist = [w for w in waits if not any(w is k for k in keep)]
        if not hoist:
            return
        for w in hoist:
            PatchedTileContext._wsplit_counter += 1
            ev = mybir.InstEventSemaphore(
                name=f"I-wsplit-{PatchedTileContext._wsplit_counter}",
                engine=inst.engine,
            )
            ev.sync_info = mybir.SyncInfo(on_wait=[w], on_update=[])
            self.nc.register_instruction(ev)
            original_block.add_instruction(ev)
        inst.sync_info = mybir.SyncInfo(on_wait=keep, on_update=list(si.on_update))

    def _commit_and_lower(self, inst, original_block, old_bb_map, bb_to_exit_bb):
        if isinstance(inst, mybir.InstLdweights):
            si = inst.sync_info
            sync_free = si is None or (not si.on_wait and not si.on_update)
            key = str(inst.ins[0]) if inst.ins else None
            if (
                sync_free
                and key is not None
                and key == self._last_pe_weights
            ):
                self.n_ldw_dropped += 1
                return  # weights already resident in the PE array
            if key is not None and sync_free:
                self._last_pe_weights = key
            else:
                self._last_pe_weights = None
        elif isinstance(inst, mybir.InstMatmult):
            if getattr(inst, "is_transpose", False):
                # transpose-mode streams its input through the weight path
                self._last_pe_weights = None
        self._split_excess_waits(inst, original_block)
        return super()._commit_and_lower(inst, original_block, old_bb_map, bb_to_exit_bb)

    def _drain_and_barrier(self, tick_clock, wait_clock):
        probe = mybir.InstNoOp(name="I-tailprobe", engine=mybir.EngineType.SP)
        wait_clock.add_sem_waits(probe, ScopedClock({None: tick_clock.global_clock}))
        waits = probe.sync_info.on_wait if probe.sync_info else []
        allocated = self.sems.allocated()
        by_name = {}
        for key, h in allocated.items():
            by_name[str(key)] = h
            name = getattr(h, "name", None)
            if name is not None:
                by_name[str(name)] = h
        for w in waits:
            h = by_name.get(w.ant_name)
            assert h is not None, (w.ant_name, list(by_name)[:40])
            self.nc.sync.wait_ge(h, w.wait_value)
        self.nc.sync.drain()
        self.nc.all_engine_barrier()
        assert self.sems is not None
        popped = self.nc._tile_sem_poison_stack.pop()
        assert popped is self._sem_poison
        self.nc.clear_and_free_semaphores(list(allocated.values()))
        self.nc.all_engine_barrier()


def build_kernel() -> bass.Bass:
    nc = bass.Bass("TRN2", target_bir_lowering=False, debug=False)
    xs = nc.dram_tensor("xs", [PB, A, D], F32, kind="ExternalInput").ap()
    # wci rows 0:512 = wcat ([W1 | W2^T | 0] padded to 128 cols for FWL),
    # rows 512:640 = the 128x128 identity (transpose operand).
    wci = nc.dram_tensor("wci", [D + P, P], DT, kind="ExternalInput").ap()
    out = nc.dram_tensor("out", [PB, A, D], F32, kind="ExternalOutput").ap()

    Exp = mybir.ActivationFunctionType.Exp
    Copy = mybir.ActivationFunctionType.Copy

    with PatchedTileContext(nc) as tc:
        with (
            tc.tile_pool(name="consts", bufs=1) as consts,
            tc.tile_pool(name="xpool", bufs=1) as xpool,
            tc.tile_pool(name="xtpool", bufs=1) as xtpool,
            tc.tile_pool(name="lrpool", bufs=1) as lrpool,
            tc.tile_pool(name="ptpool", bufs=52) as ptpool,
            tc.tile_pool(name="smpool", bufs=4) as smpool,
            tc.tile_pool(name="outpool", bufs=3) as outpool,
            # one global PSUM pool; tags totalling 8 banks:
            #   st   [128,2,512] f32 x2 = 4 banks  (score ct-pairs)
            #   pv   [128, 512] f32 x2  = 2 banks  (PV out; warmup+proj reuse)
            #   sums [128,   1] f32 x2  = 2 banks  (PV sumexp; transposes reuse)
            tc.tile_pool(name="ps", bufs=2, space="PSUM") as ps,
        ):
            # ---- consts via one early DMA on the sync queue ----
            wcat_sb = consts.tile([P, DC, P], DT)
            ident = consts.tile([P, P], DT)
            nc.sync.dma_start(wcat_sb[:], wci[0:D].rearrange("(k p) m -> p k m", p=P))
            nc.sync.dma_start(ident[:], wci[D : D + P])
            ones_dt = consts.tile([P, 1], DT)
            nc.vector.memset(ones_dt[:], 1.0)

            # left/right score operands, allocated upfront so the zero fill
            # of right rows R.. can run early (kills the rightT garbage rows
            # of left during the K=128-padded score matmuls).
            lr = {}
            for b in range(PB):
                lr[b] = (
                    lrpool.tile([P, A], DT, name=f"left_{b}"),
                    lrpool.tile([P, A], DT, name=f"right_{b}"),
                )

            # ---- PE/HAM warm-up off the wcat DMA (earliest possible dep) ----
            wps = ps.tile([P, DC * P], F32, tag="pv", name="warm_ps")
            wrhs = wcat_sb[:].rearrange("p k m -> p (k m)")
            for _ in range(WARMUP_MMS):
                nc.tensor.matmul(wps[:], lhsT=wcat_sb[:, 0, :], rhs=wrhs, start=True, stop=True)

            # ---- x loads (cast f32 -> bf16 during DMA) ----
            # First two b0 tiles ride the sync queue (it boots ~3us before
            # the compute engines); the rest go through gpsimd DGE.
            x_tiles = []
            for b in range(PB):
                x_sb = xpool.tile([P, AT, D], DT, name=f"x_{b}")
                xr = xs[b].rearrange("(t p) d -> p t d", p=P)
                if b == 0:
                    chunks = [(0, 1), (1, 1), (2, 2), (4, 4), (8, 4), (12, 4)]
                else:
                    chunks = [(0, 2), (2, 2), (4, 4), (8, 4), (12, 4)]
                for lo, ln in chunks:
                    nc.gpsimd.dma_start(x_sb[:, lo : lo + ln, :], xr[:, lo : lo + ln, :])
                x_tiles.append(x_sb)
            # b1's right-zero fill on gpsimd after its x DMAs are queued
            nc.gpsimd.memset(lr[1][1][:], 0.0)

            xt_tiles = {}
            pts = {0: {}, 1: {}}

            # ---- step generators; emission order = per-engine program order ----

            def p1_steps(b):
                """4 groups of [4 transpose-tile steps, 1 projection-chunk step]."""

                def tr_step(t):
                    def go():
                        x_sb = x_tiles[b]
                        if t == 0:
                            xt_tiles[b] = xtpool.tile([P, DC, A], DT, tag="xt", name=f"xt_{b}")
                        tr = ps.tile([P, DC, P], DT, tag="sums", name=f"tr_{b}_{t}")
                        for dc in range(DC):
                            nc.tensor.transpose(
                                tr[:, dc, :], x_sb[:, t, dc * P : (dc + 1) * P], ident[:]
                            )
                        nc.vector.tensor_copy(xt_tiles[b][:, :, t * P : (t + 1) * P], tr[:])
                    return go

                def ms0():
                    # b0's right-zero fill on the vector queue, after the
                    # first 4 transpose copies (keeps them off the critical
                    # path to projection chunk 0).
                    nc.vector.memset(lr[0][1][:], 0.0)

                def pc_step(n4):
                    def go():
                        left_sb, right_sb = lr[b]
                        direct_right = b == 0 and n4 == 0
                        if direct_right:
                            # batch 0's first score matmul is on the critical
                            # path: produce right cols 0:512 straight from a
                            # second M=10 group instead of waiting for the
                            # row-shift DMA.
                            prd = ps.tile([R, 512], F32, tag="pv", name="prd_0")
                            for dc in range(DC):
                                nc.tensor.matmul(
                                    prd[:],
                                    lhsT=wcat_sb[:, dc, R : 2 * R],
                                    rhs=xt_tiles[b][:, dc, 0:512],
                                    start=(dc == 0),
                                    stop=(dc == DC - 1),
                                )
                            nc.scalar.copy(right_sb[0:R, 0:512], prd[:])
                        # M=128 projection chunk (cols 0-9 leftT, 10-19 right,
                        # 20-127 zeros -> left rows 20.. become exact zeros).
                        pchunk = ps.tile([P, 512], F32, tag="pv", name=f"prj_{b}_{n4}")
                        for dc in range(DC):
                            nc.tensor.matmul(
                                pchunk[:],
                                lhsT=wcat_sb[:, dc, :],
                                rhs=xt_tiles[b][:, dc, n4 * 512 : (n4 + 1) * 512],
                                start=(dc == 0),
                                stop=(dc == DC - 1),
                            )
                        sl = slice(n4 * 512, (n4 + 1) * 512)
                        nc.scalar.copy(left_sb[:, sl], pchunk[:])
                        # right rows (10-19) -> partitions 0-9 via SBUF->SBUF DMA
                        if not direct_right:
                            nc.sync.dma_start(right_sb[0:R, sl], left_sb[R : 2 * R, sl])
                    return go

                steps = []
                for n4 in range(4):
                    steps += [tr_step(4 * n4 + j) for j in range(4)]
                    if b == 0 and n4 == 0:
                        steps.append(ms0)
                    steps.append(pc_step(n4))
                return steps

            def st_steps(b):
                """(a-slice q, ct-pair cg) score steps, ordered by the x/proj
                chunk each needs (chunk n unlocks max(q, cg//2) == n)."""

                def st_step(q, cg):
                    def go():
                        left_sb, right_sb = lr[b]
                        st = ps.tile([P, 2, 512], F32, tag="st", name=f"st_{b}_{q}_{cg}")
                        for u in range(2):
                            ct = 2 * cg + u
                            nc.tensor.matmul(
                                st[:, u, :],
                                lhsT=right_sb[:, ct * P : (ct + 1) * P],
                                rhs=left_sb[:, q * 512 : (q + 1) * 512],
                                start=True,
                                stop=True,
                            )
                        pt = ptpool.tile([P, 2, 512], DT, tag="pt", name=f"pt_{b}_{q}_{cg}")
                        nc.scalar.activation(
                            pt[:].rearrange("p u n -> p (u n)"),
                            st[:].rearrange("p u n -> p (u n)"),
                            Exp,
                            scale=SCALE,
                        )
                        pts[b][(q, cg)] = pt
                    return go

                steps = []
                for n in range(4):
                    for q in range(4):
                        for cg in range(8):
                            if max(q, cg // 2) == n:
                                steps.append(st_step(q, cg))
                return steps

            def pv_steps(b):
                def pv_step(at):
                    def go():
                        x_sb = x_tiles[b]
                        q, j = at // 4, at % 4
                        ops = ps.tile([P, D], F32, tag="pv", name=f"ov_{b}_{at}")
                        sums = ps.tile([P, 1], F32, tag="sums", name=f"sm_{b}_{at}")
                        for ct in range(CT):
                            cg, u = ct // 2, ct % 2
                            w = pts[b][(q, cg)][:, u, j * P : (j + 1) * P]
                            # sums first: its reused-weights matmul is nearly
                            # free and lets the next LDWEIGHTS prefetch under
                            # the N=512 matmul.
                            nc.tensor.matmul(
                                sums[:], lhsT=w, rhs=ones_dt[:],
                                start=(ct == 0), stop=(ct == CT - 1),
                            )
                            nc.tensor.matmul(
                                ops[:], lhsT=w, rhs=x_sb[:, ct, :],
                                start=(ct == 0), stop=(ct == CT - 1),
                            )
                        recip = smpool.tile([P, 1], F32, tag="recip", name=f"rc_{b}_{at}")
                        nc.vector.reciprocal(recip[:], sums[:])
                        o_sb = outpool.tile([P, D], F32, tag="o", name=f"o_{b}_{at}")
                        half = D // 2
                        orow = out[b, at * P : (at + 1) * P, :]
                        nc.vector.tensor_scalar_mul(o_sb[:, 0:half], ops[:, 0:half], recip[:])
                        nc.sync.dma_start(orow[:, 0:half], o_sb[:, 0:half])
                        nc.scalar.activation(o_sb[:, half:D], ops[:, half:D], Copy, scale=recip[:])
                        nc.sync.dma_start(orow[:, half:D], o_sb[:, half:D])
                    return go

                return [pv_step(at) for at in range(AT)]

            A_ = p1_steps(0)   # 21 steps: 4x(4 tr + proj) + ms0
            SB = st_steps(0)   # 32
            C_ = p1_steps(1)   # 20
            D_ = pv_steps(0)   # 16
            E_ = st_steps(1)   # 32
            F_ = pv_steps(1)   # 16

            # head: b0 transposes/projections with score steps threaded in as
            # their inputs land (chunk n unlocks SB prefix 2/8/18/32)
            for s in A_[0:6]:       # tr0-3, ms0, proj0
                s()
            SB[0]();  SB[1]()
            for s in A_[6:11]:      # tr4-7, proj1
                s()
            for s in SB[2:5]:
                s()
            for s in A_[11:16]:     # tr8-11, proj2
                s()
            for s in SB[5:9]:
                s()
            for s in A_[16:21]:     # tr12-15, proj3
                s()
            # remaining b0 scores threaded 1:1-ish with b1 transposes/projections
            rem = list(SB[9:])      # 23
            for i, c in enumerate(C_):
                c()
                if rem:
                    rem.pop(0)()
                if i % 2 == 0 and rem:
                    rem.pop(0)()
            while rem:
                rem.pop(0)()
            # b0 PV with b1 scores threaded 2 per step
            for i, s in enumerate(D_):
                s()
                E_[2 * i]()
                E_[2 * i + 1]()
            # b1 PV
            for s in F_:
                s()
    return nc


_NC_CACHE = None


def _get_nc():
    global _NC_CACHE
    if _NC_CACHE is None:
        _NC_CACHE = build_kernel()
    return _NC_CACHE


def make_in_maps(inputs):
    x = np.ascontiguousarray(np.asarray(inputs["x"], dtype=np.float32))
    W1 = np.asarray(inputs["W1"], dtype=np.float32)
    W2 = np.asarray(inputs["W2"], dtype=np.float32)
    wci = np.zeros((D + P, P), dtype=np.float32)
    wci[0:D, 0:R] = W1
    wci[0:D, R : 2 * R] = W2.T
    wci[D : D + P, 0:P] = np.eye(P, dtype=np.float32)
    wci = np.ascontiguousarray(wci.astype(NP_DT))
    return [
        {"xs": x[i * PB : (i + 1) * PB], "wci": wci} for i in range(NCORES)
    ]


def run(inputs, trace: bool = False):
    """Shard, execute on 8 cores, gather. Returns (out, BassKernelResults)."""
    nc = _get_nc()
    in_maps = make_in_maps(inputs)
    try:
        res = run_bass_kernel_spmd(nc, in_maps, core_ids=list(range(NCORES)), trace=trace)
    except Exception:
        # transient device hiccups (e.g. a wedged core from a prior run)
        # usually clear on retry
        res = run_bass_kernel_spmd(nc, in_maps, core_ids=list(range(NCORES)), trace=trace)
    full = np.concatenate([res.results[i]["out"] for i in range(NCORES)], axis=0)
    return full, res


def kernel(x, W1, W2):
    out, _ = run({"x": x, "W1": W1, "W2": W2})
    return out


# revision 9
# speedup vs baseline: 1.0228x; 1.0228x over previous
"""Trainium2 Bass kernel for nn_Attention_9689446220043.

Computation (per batch b):
    left  = x @ W1            [A, R]
    right = W2 @ x^T          [R, A]
    S     = left @ right      [A, A]
    P     = softmax(S / sqrt(512), axis=-1)
    out   = P @ x             [A, D]

Strategy (8 NeuronCores, data-parallel over batch B=16 -> 2 batches/core):
  - Work in the *transposed* score layout S^T[c, a] so the PV matmul
    (out = P @ x) needs no transpose of P: out[a-tile] = P^T[:, a-slice].T @ x.
  - softmax without max-subtraction (scores/sqrt(512) is within [-1.5, 1.5]
    for randn inputs, exp is safe) and with *deferred* normalization:
    out = (expS^T).T @ x, then divide rows by sumexp.
  - sumexp folded into the PV loop as N=1 matmuls reusing the PV weights
    (duplicate LDWEIGHTS elided by a custom pass), issued *before* the
    N=512 matmul of each step so the next LDWEIGHTS prefetches under the
    long matmul.
  - All matmul operands bf16 (PE streams 1 col/cycle; fp32 would be 4x),
    accumulation fp32 in PSUM. Projection weights zero-padded to M=128
    columns so fast-weight-load kicks in.
  - Head: warmup matmuls feed off the wcat DMA (sync queue starts ~3us
    before the compute engines), identity comes from DRAM, first x tiles
    ride the sync queue, and score steps are (a-slice, ct-pair) granular
    so scoring starts as soon as projection chunk 0 lands.
  - Tail: normalization split across Vector/Scalar halves with split
    output DMAs.
"""

import sys

if "/opt/trn_rl_repo" not in sys.path:
    sys.path.insert(0, "/opt/trn_rl_repo")

import ml_dtypes
import numpy as np

import concourse.bass as bass
import concourse.tile as tile
from concourse import mybir
from concourse.bass_utils import run_bass_kernel_spmd
from concourse.vector_clock import ScopedClock

# Problem shape (hardcoded per contract).
B, A, D, R = 16, 2048, 512, 10
NCORES = 8
PB = B // NCORES  # batches per core
P = 128
AT = A // P  # a-tiles (16)
CT = A // P  # c-tiles (16)
DC = D // P  # d-chunks (4)
SCALE = float(1.0 / np.sqrt(512.0))

F32 = mybir.dt.float32
DT = mybir.dt.bfloat16
NP_DT = ml_dtypes.bfloat16

WARMUP_MMS = 8


class PatchedTileContext(tile.TileContext):
    """Two fixes for this container's walrus build / perf:

    1. walrus rejects instructions carrying more than one semaphore
       sync-wait ("Too many sync wait commands"), and rejects ge-mode waits
       on InstDrain entirely. Hoist excess waits onto standalone
       EventSemaphore (wait) instructions emitted just before the owning
       instruction on the same engine.

    2. Tile splits every matmul into LDWEIGHTS+MATMUL and never dedups;
       walrus ldw-opt is disabled in this toolchain. Drop an LDWEIGHTS that
       reloads exactly the weights already in the PE array (sync-free ones
       only), so back-to-back matmuls sharing lhsT pay one weight load.
    """

    _wsplit_counter = 0

    def __init__(self, *args, **kwargs):
        super().__init__(*args, **kwargs)
        self._last_pe_weights = None
        self.n_ldw_dropped = 0

    def _split_excess_waits(self, inst, original_block):
        si = inst.sync_info
        if si is None:
            return
        waits = list(si.on_wait)
        if isinstance(inst, (mybir.InstDrain, mybir.InstNoOp)):
            keep = [w for w in waits if w.wait_mode == "sem-eq-imm"][:1]
        else:
            keep = waits[-1:]
        hoist = [w for w in waits if not any(w is k for k in keep)]
        if not hoist:
            return
        for w in hoist:
            PatchedTileContext._wsplit_counter += 1
            ev = mybir.InstEventSemaphore(
                name=f"I-wsplit-{PatchedTileContext._wsplit_counter}",
                engine=inst.engine,
            )
            ev.sync_info = mybir.SyncInfo(on_wait=[w], on_update=[])
            self.nc.register_instruction(ev)
            original_block.add_instruction(ev)
        inst.sync_info = mybir.SyncInfo(on_wait=keep, on_update=list(si.on_update))

    def _commit_and_lower(self, inst, original_block, old_bb_map, bb_to_exit_bb):
        if isinstance(inst, mybir.InstLdweights):
            si = inst.sync_info
            sync_free = si is None or (not si.on_wait and not si.on_update)
            key = str(inst.ins[0]) if inst.ins else None
            if (
                sync_free
                and key is not None
                and key == self._last_pe_weights
            ):
                self.n_ldw_dropped += 1
                return  # weights already resident in the PE array
            if key is not None and sync_free:
                self._last_pe_weights = key
            else:
                self._last_pe_weights = None
        elif isinstance(inst, mybir.InstMatmult):
            if getattr(inst, "is_transpose", False):
                # transpose-mode streams its input through the weight path
                self._last_pe_weights = None
        self._split_excess_waits(inst, original_block)
        return super()._commit_and_lower(inst, original_block, old_bb_map, bb_to_exit_bb)

    def _drain_and_barrier(self, tick_clock, wait_clock):
        probe = mybir.InstNoOp(name="I-tailprobe", engine=mybir.EngineType.SP)
        wait_clock.add_sem_waits(probe, ScopedClock({None: tick_clock.global_clock}))
        waits = probe.sync_info.on_wait if probe.sync_info else []
        allocated = self.sems.allocated()
        by_name = {}
        for key, h in allocated.items():
            by_name[str(key)] = h
            name = getattr(h, "name", None)
            if name is not None:
                by_name[str(name)] = h
        for w in waits:
            h = by_name.get(w.ant_name)
            assert h is not None, (w.ant_name, list(by_name)[:40])
            self.nc.sync.wait_ge(h, w.wait_value)
        self.nc.sync.drain()
        self.nc.all_engine_barrier()
        assert self.sems is not None
        popped = self.nc._tile_sem_poison_stack.pop()
        assert popped is self._sem_poison
        self.nc.clear_and_free_semaphores(list(allocated.values()))
        self.nc.all_engine_barrier()


def build_kernel() -> bass.Bass:
    nc = bass.Bass("TRN2", target_bir_lowering=False, debug=False)
    xs = nc.dram_tensor("xs", [PB, A, D], F32, kind="ExternalInput").ap()
    # wci rows 0:512 = wcat ([W1 | W2^T | 0] padded to 128 cols for FWL),
    # rows 512:640 = the 128x128 identity (transpose operand).
    wci = nc.dram_tensor("wci", [D + P, P], DT, kind="ExternalInput").ap()
    out = nc.dram_tensor("out", [PB, A, D], F32, kind="ExternalOutput").ap()

    Exp = mybir.ActivationFunctionType.Exp
    Copy = mybir.ActivationFunctionType.Copy

    with PatchedTileContext(nc) as tc:
        with (
            tc.tile_pool(name="consts", bufs=1) as consts,
            tc.tile_pool(name="xpool", bufs=1) as xpool,
            tc.tile_pool(name="xtpool", bufs=1) as xtpool,
            tc.tile_pool(name="lrpool", bufs=1) as lrpool,
            tc.tile_pool(name="ptpool", bufs=52) as ptpool,
            tc.tile_pool(name="smpool", bufs=4) as smpool,
            tc.tile_pool(name="outpool", bufs=3) as outpool,
            # one global PSUM pool; tags totalling 8 banks:
            #   st   [128,2,512] f32 x2 = 4 banks  (score ct-pairs)
            #   pv   [128, 512] f32 x2  = 2 banks  (PV out; warmup+proj reuse)
            #   sums [128,   1] f32 x2  = 2 banks  (PV sumexp; transposes reuse)
            tc.tile_pool(name="ps", bufs=2, space="PSUM") as ps,
        ):
            # ---- consts via one early DMA on the sync queue ----
            wcat_sb = consts.tile([P, DC, P], DT)
            ident = consts.tile([P, P], DT)
            nc.sync.dma_start(wcat_sb[:], wci[0:D].rearrange("(k p) m -> p k m", p=P))
            nc.sync.dma_start(ident[:], wci[D : D + P])
            ones_dt = consts.tile([P, 1], DT)
            nc.vector.memset(ones_dt[:], 1.0)

            # left/right score operands, allocated upfront so the zero fill
            # of right rows R.. can run early (kills the rightT garbage rows
            # of left during the K=128-padded score matmuls).
            lr = {}
            for b in range(PB):
                lr[b] = (
                    lrpool.tile([P, A], DT, name=f"left_{b}"),
                    lrpool.tile([P, A], DT, name=f"right_{b}"),
                )

            # ---- PE/HAM warm-up off the wcat DMA (earliest possible dep) ----
            wps = ps.tile([P, DC * P], F32, tag="pv", name="warm_ps")
            wrhs = wcat_sb[:].rearrange("p k m -> p (k m)")
            for _ in range(WARMUP_MMS):
                nc.tensor.matmul(wps[:], lhsT=wcat_sb[:, 0, :], rhs=wrhs, start=True, stop=True)

            # ---- x loads (cast f32 -> bf16 during DMA) ----
            # First two b0 tiles ride the sync queue (it boots ~3us before
            # the compute engines); the rest go through gpsimd DGE.
            x_tiles = []
            for b in range(PB):
                x_sb = xpool.tile([P, AT, D], DT, name=f"x_{b}")
                xr = xs[b].rearrange("(t p) d -> p t d", p=P)
                if b == 0:
                    chunks = [(0, 1), (1, 1), (2, 2), (4, 4), (8, 4), (12, 4)]
                else:
                    chunks = [(0, 2), (2, 2), (4, 4), (8, 4), (12, 4)]
                for lo, ln in chunks:
                    nc.gpsimd.dma_start(x_sb[:, lo : lo + ln, :], xr[:, lo : lo + ln, :])
                x_tiles.append(x_sb)
            # b1's right-zero fill on gpsimd after its x DMAs are queued
            nc.gpsimd.memset(lr[1][1][:], 0.0)

            xt_tiles = {}
            pts = {0: {}, 1: {}}

            # ---- step generators; emission order = per-engine program order ----

            def p1_steps(b):
                """4 groups of [4 transpose-tile steps, 1 projection-chunk step]."""

                def tr_step(t):
                    def go():
                        x_sb = x_tiles[b]
                        if t == 0:
                            xt_tiles[b] = xtpool.tile([P, DC, A], DT, tag="xt", name=f"xt_{b}")
                        tr = ps.tile([P, DC, P], DT, tag="sums", name=f"tr_{b}_{t}")
                        for dc in range(DC):
                            nc.tensor.transpose(
                                tr[:, dc, :], x_sb[:, t, dc * P : (dc + 1) * P], ident[:]
                            )
                        nc.vector.tensor_copy(xt_tiles[b][:, :, t * P : (t + 1) * P], tr[:])
                    return go

                def ms0():
                    # b0's right-zero fill on the vector queue, after the
                    # first 4 transpose copies (keeps them off the critical
                    # path to projection chunk 0).
                    nc.vector.memset(lr[0][1][:], 0.0)

                def pc_step(n4):
                    def go():
                        left_sb, right_sb = lr[b]
                        direct_right = b == 0 and n4 == 0
                        if direct_right:
                            # batch 0's first score matmul is on the critical
                            # path: produce right cols 0:512 straight from a
                            # second M=10 group instead of waiting for the
                            # row-shift DMA.
                            prd = ps.tile([R, 512], F32, tag="pv", name="prd_0")
                            for dc in range(DC):
                                nc.tensor.matmul(
                                    prd[:],
                                    lhsT=wcat_sb[:, dc, R : 2 * R],
                                    rhs=xt_tiles[b][:, dc, 0:512],
                                    start=(dc == 0),
                                    stop=(dc == DC - 1),
                                )
                            nc.scalar.copy(right_sb[0:R, 0:512], prd[:])
                        # M=128 projection chunk (cols 0-9 leftT, 10-19 right,
                        # 20-127 zeros -> left rows 20.. become exact zeros).
                        pchunk = ps.tile([P, 512], F32, tag="pv", name=f"prj_{b}_{n4}")
                        for dc in range(DC):
                            nc.tensor.matmul(
                                pchunk[:],
                                lhsT=wcat_sb[:, dc, :],
                                rhs=xt_tiles[b][:, dc, n4 * 512 : (n4 + 1) * 512],
                                start=(dc == 0),
                                stop=(dc == DC - 1),
                            )
                        sl = slice(n4 * 512, (n4 + 1) * 512)
                        nc.scalar.copy(left_sb[:, sl], pchunk[:])
                        # right rows (10-19) -> partitions 0-9 via SBUF->SBUF DMA
                        if not direct_right:
                            nc.sync.dma_start(right_sb[0:R, sl], left_sb[R : 2 * R, sl])
                    return go

                steps = []
                for n4 in range(4):
                    steps += [tr_step(4 * n4 + j) for j in range(4)]
                    if b == 0 and n4 == 0:
                        steps.append(ms0)
                    steps.append(pc_step(n4))
                return steps

            def st_steps(b):
                """(a-slice q, ct-pair cg) score steps, ordered by the x/proj
                chunk each needs (chunk n unlocks max(q, cg//2) == n)."""

                def st_step(q, cg):
                    def go():
                        left_sb, right_sb = lr[b]
                        st = ps.tile([P, 2, 512], F32, tag="st", name=f"st_{b}_{q}_{cg}")
                        for u in range(2):
                            ct = 2 * cg + u
                            nc.tensor.matmul(
                                st[:, u, :],
                                lhsT=right_sb[:, ct * P : (ct + 1) * P],
                                rhs=left_sb[:, q * 512 : (q + 1) * 512],
                                start=True,
                                stop=True,
                            )
                        pt = ptpool.tile([P, 2, 512], DT, tag="pt", name=f"pt_{b}_{q}_{cg}")
                        nc.scalar.activation(
                            pt[:].rearrange("p u n -> p (u n)"),
                            st[:].rearrange("p u n -> p (u n)"),
                            Exp,
                            scale=SCALE,
                        )
                        pts[b][(q, cg)] = pt
                    return go

                steps = []
                for n in range(4):
                    for q in range(4):
                        for cg in range(8):
                            if max(q, cg // 2) == n:
                                steps.append(st_step(q, cg))
                return steps

            def pv_steps(b):
                def pv_step(at):
                    def go():
                        x_sb = x_tiles[b]
                        q, j = at // 4, at % 4
                        ops = ps.tile([P, D], F32, tag="pv", name=f"ov_{b}_{at}")
                        sums = ps.tile([P, 1], F32, tag="sums", name=f"sm_{b}_{at}")
                        for ct in range(CT):
                            cg, u = ct // 2, ct % 2
                            w = pts[b][(q, cg)][:, u, j * P : (j + 1) * P]
                            # sums first: its reused-weights matmul is nearly
                            # free and lets the next LDWEIGHTS prefetch under
                            # the N=512 matmul.
                            nc.tensor.matmul(
                                sums[:], lhsT=w, rhs=ones_dt[:],
                                start=(ct == 0), stop=(ct == CT - 1),
                            )
                            nc.tensor.matmul(
                                ops[:], lhsT=w, rhs=x_sb[:, ct, :],
                                start=(ct == 0), stop=(ct == CT - 1),
                            )
                        recip = smpool.tile([P, 1], F32, tag="recip", name=f"rc_{b}_{at}")
                        nc.vector.reciprocal(recip[:], sums[:])
                        o_sb = outpool.tile([P, D], F32, tag="o", name=f"o_{b}_{at}")
                        half = D // 2
                        orow = out[b, at * P : (at + 1) * P, :]
                        nc.vector.tensor_scalar_mul(o_sb[:, 0:half], ops[:, 0:half], recip[:])
                        nc.sync.dma_start(orow[:, 0:half], o_sb[:, 0:half])
                        nc.scalar.activation(o_sb[:, half:D], ops[:, half:D], Copy, scale=recip[:])
                        nc.sync.dma_start(orow[:, half:D], o_sb[:, half:D])
                    return go

                return [pv_step(at) for at in range(AT)]

            A_ = p1_steps(0)   # 21 steps: 4x(4 tr + proj) + ms0
            C_ = p1_steps(1)   # 20
            D_ = pv_steps(0)   # 16
            F_ = pv_steps(1)   # 16
            # score steps grouped by a-slice q; each group of 8 is one
            # softmax-row's worth for 512 a's
            SBq = {}
            EQ = {}
            for q0 in range(4):
                SBq[q0] = []
                EQ[q0] = []
            for s, dst in ((st_steps(0), SBq), (st_steps(1), EQ)):
                i = 0
                for n in range(4):
                    for q0 in range(4):
                        for cg in range(8):
                            if max(q0, cg // 2) == n:
                                dst[q0].append(s[i])
                                i += 1

            # head: b0 transposes/projections with q=0 score steps threaded
            # in as their inputs land (chunk n unlocks 2 of them)
            for s in A_[0:6]:       # tr0-3, ms0, proj0
                s()
            SBq[0][0](); SBq[0][1]()
            for s in A_[6:11]:      # tr4-7, proj1
                s()
            SBq[0][2](); SBq[0][3]()
            for s in A_[11:16]:     # tr8-11, proj2
                s()
            SBq[0][4](); SBq[0][5]()
            for s in A_[16:21]:     # tr12-15, proj3
                s()
            SBq[0][6](); SBq[0][7]()
            # b1 transposes/projections with b0's q=1 scores threaded in;
            # q>=2 scores move to the PV phase where the scalar engine
            # (exp) has slack
            q1 = list(SBq[1])
            for i, c in enumerate(C_):
                c()
                if i % 2 == 0 and q1:
                    q1.pop(0)()
            while q1:
                q1.pop(0)()
            # b0 PV; thread b0's remaining scores then b1's q0/q1, 2 per step
            dthread = SBq[2] + SBq[3] + EQ[0] + EQ[1]   # 32
            for i, s in enumerate(D_):
                s()
                dthread[2 * i]()
                dthread[2 * i + 1]()
            # b1 PV; thread b1's q2/q3 scores into the first half
            fthread = EQ[2] + EQ[3]   # 16
            for i, s in enumerate(F_):
                s()
                if i < 8:
                    fthread[2 * i]()
                    fthread[2 * i + 1]()
    return nc


_NC_CACHE = None


def _get_nc():
    global _NC_CACHE
    if _NC_CACHE is None:
        _NC_CACHE = build_kernel()
    return _NC_CACHE


def make_in_maps(inputs):
    x = np.ascontiguousarray(np.asarray(inputs["x"], dtype=np.float32))
    W1 = np.asarray(inputs["W1"], dtype=np.float32)
    W2 = np.asarray(inputs["W2"], dtype=np.float32)
    wci = np.zeros((D + P, P), dtype=np.float32)
    wci[0:D, 0:R] = W1
    wci[0:D, R : 2 * R] = W2.T
    wci[D : D + P, 0:P] = np.eye(P, dtype=np.float32)
    wci = np.ascontiguousarray(wci.astype(NP_DT))
    return [
        {"xs": x[i * PB : (i + 1) * PB], "wci": wci} for i in range(NCORES)
    ]


def run(inputs, trace: bool = False):
    """Shard, execute on 8 cores, gather. Returns (out, BassKernelResults)."""
    nc = _get_nc()
    in_maps = make_in_maps(inputs)
    try:
        res = run_bass_kernel_spmd(nc, in_maps, core_ids=list(range(NCORES)), trace=trace)
    except Exception:
        # transient device hiccups (e.g. a wedged core from a prior run)
        # usually clear on retry
        res = run_bass_kernel_spmd(nc, in_maps, core_ids=list(range(NCORES)), trace=trace)
    full = np.concatenate([res.results[i]["out"] for i in range(NCORES)], axis=0)
    return full, res


def kernel(x, W1, W2):
    out, _ = run({"x": x, "W1": W1, "W2": W2})
    return out


# revision 12
# speedup vs baseline: 1.0314x; 1.0084x over previous
"""Trainium2 Bass kernel for nn_Attention_9689446220043.

Computation (per batch b):
    left  = x @ W1            [A, R]
    right = W2 @ x^T          [R, A]
    S     = left @ right      [A, A]
    P     = softmax(S / sqrt(512), axis=-1)
    out   = P @ x             [A, D]

Strategy (8 NeuronCores, data-parallel over batch B=16 -> 2 batches/core):
  - Work in the *transposed* score layout S^T[c, a] so the PV matmul
    (out = P @ x) needs no transpose of P: out[a-tile] = P^T[:, a-slice].T @ x.
  - softmax without max-subtraction (scores/sqrt(512) is within [-1.5, 1.5]
    for randn inputs, exp is safe) and with *deferred* normalization:
    out = (expS^T).T @ x, then divide rows by sumexp.
  - sumexp folded into the PV loop as N=1 matmuls reusing the PV weights
    (duplicate LDWEIGHTS elided by a custom pass), issued *before* the
    N=512 matmul of each step so the next LDWEIGHTS prefetches under the
    long matmul.
  - All matmul operands bf16 (PE streams 1 col/cycle; fp32 would be 4x),
    accumulation fp32 in PSUM. Projection weights zero-padded to M=128
    columns so fast-weight-load kicks in.
  - Head: warmup matmuls feed off the wcat DMA (sync queue starts ~3us
    before the compute engines), identity comes from DRAM, first x tiles
    ride the sync queue, and score steps are (a-slice, ct-pair) granular
    so scoring starts as soon as projection chunk 0 lands.
  - Tail: normalization split across Vector/Scalar halves with split
    output DMAs.
"""

import sys

if "/opt/trn_rl_repo" not in sys.path:
    sys.path.insert(0, "/opt/trn_rl_repo")

import ml_dtypes
import numpy as np

import concourse.bass as bass
import concourse.tile as tile
from concourse import mybir
from concourse.bass_utils import run_bass_kernel_spmd
from concourse.vector_clock import ScopedClock

# Problem shape (hardcoded per contract).
B, A, D, R = 16, 2048, 512, 10
NCORES = 8
PB = B // NCORES  # batches per core
P = 128
AT = A // P  # a-tiles (16)
CT = A // P  # c-tiles (16)
DC = D // P  # d-chunks (4)
SCALE = float(1.0 / np.sqrt(512.0))

F32 = mybir.dt.float32
DT = mybir.dt.bfloat16
NP_DT = ml_dtypes.bfloat16

WARMUP_MMS = 8


class PatchedTileContext(tile.TileContext):
    """Two fixes for this container's walrus build / perf:

    1. walrus rejects instructions carrying more than one semaphore
       sync-wait ("Too many sync wait commands"), and rejects ge-mode waits
       on InstDrain entirely. Hoist excess waits onto standalone
       EventSemaphore (wait) instructions emitted just before the owning
       instruction on the same engine.

    2. Tile splits every matmul into LDWEIGHTS+MATMUL and never dedups;
       walrus ldw-opt is disabled in this toolchain. Drop an LDWEIGHTS that
       reloads exactly the weights already in the PE array (sync-free ones
       only), so back-to-back matmuls sharing lhsT pay one weight load.
    """

    _wsplit_counter = 0

    def __init__(self, *args, **kwargs):
        super().__init__(*args, **kwargs)
        self._last_pe_weights = None
        self.n_ldw_dropped = 0

    def _split_excess_waits(self, inst, original_block):
        si = inst.sync_info
        if si is None:
            return
        waits = list(si.on_wait)
        if isinstance(inst, (mybir.InstDrain, mybir.InstNoOp)):
            keep = [w for w in waits if w.wait_mode == "sem-eq-imm"][:1]
        else:
            keep = waits[-1:]
        hoist = [w for w in waits if not any(w is k for k in keep)]
        if not hoist:
            return
        for w in hoist:
            PatchedTileContext._wsplit_counter += 1
            ev = mybir.InstEventSemaphore(
                name=f"I-wsplit-{PatchedTileContext._wsplit_counter}",
                engine=inst.engine,
            )
            ev.sync_info = mybir.SyncInfo(on_wait=[w], on_update=[])
            self.nc.register_instruction(ev)
            original_block.add_instruction(ev)
        inst.sync_info = mybir.SyncInfo(on_wait=keep, on_update=list(si.on_update))

    def _commit_and_lower(self, inst, original_block, old_bb_map, bb_to_exit_bb):
        if isinstance(inst, mybir.InstLdweights):
            si = inst.sync_info
            sync_free = si is None or (not si.on_wait and not si.on_update)
            key = str(inst.ins[0]) if inst.ins else None
            if (
                sync_free
                and key is not None
                and key == self._last_pe_weights
            ):
                self.n_ldw_dropped += 1
                return  # weights already resident in the PE array
            if key is not None and sync_free:
                self._last_pe_weights = key
            else:
                self._last_pe_weights = None
        elif isinstance(inst, mybir.InstMatmult):
            if getattr(inst, "is_transpose", False):
                # transpose-mode streams its input through the weight path
                self._last_pe_weights = None
        self._split_excess_waits(inst, original_block)
        return super()._commit_and_lower(inst, original_block, old_bb_map, bb_to_exit_bb)

    def _drain_and_barrier(self, tick_clock, wait_clock):
        probe = mybir.InstNoOp(name="I-tailprobe", engine=mybir.EngineType.SP)
        wait_clock.add_sem_waits(probe, ScopedClock({None: tick_clock.global_clock}))
        waits = probe.sync_info.on_wait if probe.sync_info else []
        allocated = self.sems.allocated()
        by_name = {}
        for key, h in allocated.items():
            by_name[str(key)] = h
            name = getattr(h, "name", None)
            if name is not None:
                by_name[str(name)] = h
        for w in waits:
            h = by_name.get(w.ant_name)
            assert h is not None, (w.ant_name, list(by_name)[:40])
            self.nc.sync.wait_ge(h, w.wait_value)
        self.nc.sync.drain()
        self.nc.all_engine_barrier()
        assert self.sems is not None
        popped = self.nc._tile_sem_poison_stack.pop()
        assert popped is self._sem_poison
        self.nc.clear_and_free_semaphores(list(allocated.values()))
        self.nc.all_engine_barrier()


def build_kernel() -> bass.Bass:
    nc = bass.Bass("TRN2", target_bir_lowering=False, debug=False)
    xs = nc.dram_tensor("xs", [PB, A, D], F32, kind="ExternalInput").ap()
    # wci rows 0:512 = wcat ([W1 | W2^T | 0] padded to 128 cols for FWL),
    # rows 512:640 = the 128x128 identity (transpose operand).
    wci = nc.dram_tensor("wci", [D + P, P], DT, kind="ExternalInput").ap()
    out = nc.dram_tensor("out", [PB, A, D], F32, kind="ExternalOutput").ap()

    Exp = mybir.ActivationFunctionType.Exp
    Copy = mybir.ActivationFunctionType.Copy

    with PatchedTileContext(nc) as tc:
        with (
            tc.tile_pool(name="consts", bufs=1) as consts,
            tc.tile_pool(name="xpool", bufs=1) as xpool,
            tc.tile_pool(name="xtpool", bufs=1) as xtpool,
            tc.tile_pool(name="lrpool", bufs=1) as lrpool,
            tc.tile_pool(name="ptpool", bufs=52) as ptpool,
            tc.tile_pool(name="smpool", bufs=4) as smpool,
            tc.tile_pool(name="outpool", bufs=3) as outpool,
            # one global PSUM pool; tags totalling 8 banks:
            #   st   [128,2,512] f32 x2 = 4 banks  (score ct-pairs)
            #   pv   [128, 512] f32 x2  = 2 banks  (PV out; warmup+proj reuse)
            #   sums [128,   1] f32 x2  = 2 banks  (PV sumexp; transposes reuse)
            tc.tile_pool(name="ps", bufs=2, space="PSUM") as ps,
        ):
            # ---- consts via one early DMA on the sync queue ----
            wcat_sb = consts.tile([P, DC, P], DT)
            ident = consts.tile([P, P], DT)
            nc.sync.dma_start(wcat_sb[:], wci[0:D].rearrange("(k p) m -> p k m", p=P))
            nc.sync.dma_start(ident[:], wci[D : D + P])
            junk = consts.tile([P, 512], DT)
            nc.vector.memset(junk[:], 0.0)
            ones_dt = consts.tile([P, 1], DT)
            nc.vector.memset(ones_dt[:], 1.0)
            # preload the scalar engine's activation table during the
            # DMA-bound head (the first real exp otherwise pays ~1.3us)
            preheat = smpool.tile([P, 1], F32, tag="recip", name="preheat")
            nc.scalar.activation(preheat[:], ones_dt[:], Exp, scale=1.0)

            # left/right score operands, allocated upfront so the zero fill
            # of right rows R.. can run early (kills the rightT garbage rows
            # of left during the K=128-padded score matmuls).
            lr = {}
            for b in range(PB):
                lr[b] = (
                    lrpool.tile([P, A], DT, name=f"left_{b}"),
                    lrpool.tile([P, A], DT, name=f"right_{b}"),
                )

            # ---- PE/HAM warm-up off the junk memset (ready ~1.5us before
            # the wcat DMA lands) ----
            wps = ps.tile([P, DC * P], F32, tag="pv", name="warm_ps")
            for _ in range(WARMUP_MMS):
                nc.tensor.matmul(wps[:], lhsT=junk[:, 0:P], rhs=junk[:], start=True, stop=True)

            # ---- x loads (cast f32 -> bf16 during DMA) ----
            # First two b0 tiles ride the sync queue (it boots ~3us before
            # the compute engines); the rest go through gpsimd DGE.
            x_tiles = []
            for b in range(PB):
                x_sb = xpool.tile([P, AT, D], DT, name=f"x_{b}")
                xr = xs[b].rearrange("(t p) d -> p t d", p=P)
                if b == 0:
                    chunks = [(0, 1), (1, 1), (2, 2), (4, 4), (8, 4), (12, 4)]
                else:
                    chunks = [(0, 2), (2, 2), (4, 2), (6, 2), (8, 4), (12, 4)]
                for lo, ln in chunks:
                    nc.gpsimd.dma_start(x_sb[:, lo : lo + ln, :], xr[:, lo : lo + ln, :])
                x_tiles.append(x_sb)
            # b1's right-zero fill on gpsimd after its x DMAs are queued
            nc.gpsimd.memset(lr[1][1][:], 0.0)

            xt_tiles = {}
            pts = {0: {}, 1: {}}

            # ---- step generators; emission order = per-engine program order ----

            def p1_steps(b):
                """4 groups of [4 transpose-tile steps, 1 projection-chunk step]."""

                def tr_step(t):
                    def go():
                        x_sb = x_tiles[b]
                        if t == 0:
                            xt_tiles[b] = xtpool.tile([P, DC, A], DT, tag="xt", name=f"xt_{b}")
                        tr = ps.tile([P, DC, P], DT, tag="sums", name=f"tr_{b}_{t}")
                        for dc in range(DC):
                            nc.tensor.transpose(
                                tr[:, dc, :], x_sb[:, t, dc * P : (dc + 1) * P], ident[:]
                            )
                        nc.vector.tensor_copy(xt_tiles[b][:, :, t * P : (t + 1) * P], tr[:])
                    return go

                def ms0():
                    # b0's right-zero fill on the vector queue, after the
                    # first 4 transpose copies (keeps them off the critical
                    # path to projection chunk 0).
                    nc.vector.memset(lr[0][1][:], 0.0)

                def pc_step(n4):
                    def go():
                        left_sb, right_sb = lr[b]
                        direct_right = b == 0 and n4 == 0
                        if direct_right:
                            # batch 0's first score matmul is on the critical
                            # path: produce right cols 0:512 straight from a
                            # second M=10 group instead of waiting for the
                            # row-shift DMA.
                            prd = ps.tile([R, 512], F32, tag="pv", name="prd_0")
                            for dc in range(DC):
                                nc.tensor.matmul(
                                    prd[:],
                                    lhsT=wcat_sb[:, dc, R : 2 * R],
                                    rhs=xt_tiles[b][:, dc, 0:512],
                                    start=(dc == 0),
                                    stop=(dc == DC - 1),
                                )
                            nc.scalar.copy(right_sb[0:R, 0:512], prd[:])
                        # M=128 projection chunk (cols 0-9 leftT, 10-19 right,
                        # 20-127 zeros -> left rows 20.. become exact zeros).
                        pchunk = ps.tile([P, 512], F32, tag="pv", name=f"prj_{b}_{n4}")
                        for dc in range(DC):
                            nc.tensor.matmul(
                                pchunk[:],
                                lhsT=wcat_sb[:, dc, :],
                                rhs=xt_tiles[b][:, dc, n4 * 512 : (n4 + 1) * 512],
                                start=(dc == 0),
                                stop=(dc == DC - 1),
                            )
                        sl = slice(n4 * 512, (n4 + 1) * 512)
                        nc.scalar.copy(left_sb[:, sl], pchunk[:])
                        # right rows (10-19) -> partitions 0-9 via SBUF->SBUF DMA
                        if not direct_right:
                            nc.sync.dma_start(right_sb[0:R, sl], left_sb[R : 2 * R, sl])
                    return go

                steps = []
                for n4 in range(4):
                    steps += [tr_step(4 * n4 + j) for j in range(4)]
                    if b == 0 and n4 == 0:
                        steps.append(ms0)
                    steps.append(pc_step(n4))
                return steps

            def st_steps(b):
                """(a-slice q, ct-pair cg) score steps, ordered by the x/proj
                chunk each needs (chunk n unlocks max(q, cg//2) == n)."""

                def st_step(q, cg):
                    def go():
                        left_sb, right_sb = lr[b]
                        st = ps.tile([P, 2, 512], F32, tag="st", name=f"st_{b}_{q}_{cg}")
                        for u in range(2):
                            ct = 2 * cg + u
                            nc.tensor.matmul(
                                st[:, u, :],
                                lhsT=right_sb[:, ct * P : (ct + 1) * P],
                                rhs=left_sb[:, q * 512 : (q + 1) * 512],
                                start=True,
                                stop=True,
                            )
                        pt = ptpool.tile([P, 2, 512], DT, tag="pt", name=f"pt_{b}_{q}_{cg}")
                        nc.scalar.activation(
                            pt[:].rearrange("p u n -> p (u n)"),
                            st[:].rearrange("p u n -> p (u n)"),
                            Exp,
                            scale=SCALE,
                        )
                        pts[b][(q, cg)] = pt
                    return go

                steps = []
                for n in range(4):
                    for q in range(4):
                        for cg in range(8):
                            if max(q, cg // 2) == n:
                                steps.append(st_step(q, cg))
                return steps

            def pv_steps(b):
                def pv_step(at):
                    def go():
                        x_sb = x_tiles[b]
                        q, j = at // 4, at % 4
                        ops = ps.tile([P, D], F32, tag="pv", name=f"ov_{b}_{at}")
                        sums = ps.tile([P, 1], F32, tag="sums", name=f"sm_{b}_{at}")
                        for ct in range(CT):
                            cg, u = ct // 2, ct % 2
                            w = pts[b][(q, cg)][:, u, j * P : (j + 1) * P]
                            # sums first: its reused-weights matmul is nearly
                            # free and lets the next LDWEIGHTS prefetch under
                            # the N=512 matmul.
                            nc.tensor.matmul(
                                sums[:], lhsT=w, rhs=ones_dt[:],
                                start=(ct == 0), stop=(ct == CT - 1),
                            )
                            nc.tensor.matmul(
                                ops[:], lhsT=w, rhs=x_sb[:, ct, :],
                                start=(ct == 0), stop=(ct == CT - 1),
                            )
                        recip = smpool.tile([P, 1], F32, tag="recip", name=f"rc_{b}_{at}")
                        nc.vector.reciprocal(recip[:], sums[:])
                        o_sb = outpool.tile([P, D], F32, tag="o", name=f"o_{b}_{at}")
                        half = D // 2
                        orow = out[b, at * P : (at + 1) * P, :]
                        nc.vector.tensor_scalar_mul(o_sb[:, 0:half], ops[:, 0:half], recip[:])
                        nc.sync.dma_start(orow[:, 0:half], o_sb[:, 0:half])
                        nc.scalar.activation(o_sb[:, half:D], ops[:, half:D], Copy, scale=recip[:])
                        nc.sync.dma_start(orow[:, half:D], o_sb[:, half:D])
                    return go

                return [pv_step(at) for at in range(AT)]

            A_ = p1_steps(0)   # 21 steps: 4x(4 tr + proj) + ms0
            C_ = p1_steps(1)   # 20
            D_ = pv_steps(0)   # 16
            F_ = pv_steps(1)   # 16
            # score steps grouped by a-slice q; each group of 8 is one
            # softmax-row's worth for 512 a's
            SBq = {}
            EQ = {}
            for q0 in range(4):
                SBq[q0] = []
                EQ[q0] = []
            for s, dst in ((st_steps(0), SBq), (st_steps(1), EQ)):
                i = 0
                for n in range(4):
                    for q0 in range(4):
                        for cg in range(8):
                            if max(q0, cg // 2) == n:
                                dst[q0].append(s[i])
                                i += 1

            # head: b0 transposes/projections with q=0 score steps threaded
            # in as their inputs land (chunk n unlocks 2 of them)
            for s in A_[0:6]:       # tr0-3, ms0, proj0
                s()
            SBq[0][0](); SBq[0][1]()
            for s in A_[6:11]:      # tr4-7, proj1
                s()
            SBq[0][2](); SBq[0][3]()
            for s in A_[11:16]:     # tr8-11, proj2
                s()
            SBq[0][4](); SBq[0][5]()
            for s in A_[16:21]:     # tr12-15, proj3
                s()
            SBq[0][6](); SBq[0][7]()
            # b1 transposes/projections with b0's q=1 scores threaded in;
            # q>=2 scores move to the PV phase where the scalar engine
            # (exp) has slack
            q1 = list(SBq[1])
            for i, c in enumerate(C_):
                c()
                if i % 2 == 0 and q1:
                    q1.pop(0)()
            while q1:
                q1.pop(0)()
            # b0 PV; thread b0's remaining scores then b1's q0/q1, 2 per step
            dthread = SBq[2] + SBq[3] + EQ[0] + EQ[1]   # 32
            for i, s in enumerate(D_):
                s()
                dthread[2 * i]()
                dthread[2 * i + 1]()
            # b1 PV; thread b1's q2/q3 scores into the first half
            fthread = EQ[2] + EQ[3]   # 16
            for i, s in enumerate(F_):
                s()
                if i < 8:
                    fthread[2 * i]()
                    fthread[2 * i + 1]()
    return nc


_NC_CACHE = None


def _get_nc():
    global _NC_CACHE
    if _NC_CACHE is None:
        _NC_CACHE = build_kernel()
    return _NC_CACHE


def make_in_maps(inputs):
    x = np.ascontiguousarray(np.asarray(inputs["x"], dtype=np.float32))
    W1 = np.asarray(inputs["W1"], dtype=np.float32)
    W2 = np.asarray(inputs["W2"], dtype=np.float32)
    wci = np.zeros((D + P, P), dtype=np.float32)
    wci[0:D, 0:R] = W1
    wci[0:D, R : 2 * R] = W2.T
    wci[D : D + P, 0:P] = np.eye(P, dtype=np.float32)
    wci = np.ascontiguousarray(wci.astype(NP_DT))
    return [
        {"xs": x[i * PB : (i + 1) * PB], "wci": wci} for i in range(NCORES)
    ]


def run(inputs, trace: bool = False):
    """Shard, execute on 8 cores, gather. Returns (out, BassKernelResults)."""
    nc = _get_nc()
    in_maps = make_in_maps(inputs)
    try:
        res = run_bass_kernel_spmd(nc, in_maps, core_ids=list(range(NCORES)), trace=trace)
    except Exception:
        # transient device hiccups (e.g. a wedged core from a prior run)
        # usually clear on retry
        res = run_bass_kernel_spmd(nc, in_maps, core_ids=list(range(NCORES)), trace=trace)
    full = np.concatenate([res.results[i]["out"] for i in range(NCORES)], axis=0)
    return full, res


def kernel(x, W1, W2):
    out, _ = run({"x": x, "W1": W1, "W2": W2})
    return out


# revision 15
# speedup vs baseline: 1.0349x; 1.0034x over previous
"""Trainium2 Bass kernel for nn_Attention_9689446220043.

Computation (per batch b):
    left  = x @ W1            [A, R]
    right = W2 @ x^T          [R, A]
    S     = left @ right      [A, A]
    P     = softmax(S / sqrt(512), axis=-1)
    out   = P @ x             [A, D]

Strategy (8 NeuronCores, data-parallel over batch B=16 -> 2 batches/core):
  - Work in the *transposed* score layout S^T[c, a] so the PV matmul
    (out = P @ x) needs no transpose of P: out[a-tile] = P^T[:, a-slice].T @ x.
  - softmax without max-subtraction (scores/sqrt(512) is within [-1.5, 1.5]
    for randn inputs, exp is safe) and with *deferred* normalization:
    out = (expS^T).T @ x, then divide rows by sumexp.
  - sumexp folded into the PV loop as N=1 matmuls reusing the PV weights
    (duplicate LDWEIGHTS elided by a custom pass), issued *before* the
    N=512 matmul of each step so the next LDWEIGHTS prefetches under the
    long matmul.
  - All matmul operands bf16 (PE streams 1 col/cycle; fp32 would be 4x),
    accumulation fp32 in PSUM. Projection weights zero-padded to M=128
    columns so fast-weight-load kicks in.
  - Head: warmup matmuls feed off the wcat DMA (sync queue starts ~3us
    before the compute engines), identity comes from DRAM, first x tiles
    ride the sync queue, and score steps are (a-slice, ct-pair) granular
    so scoring starts as soon as projection chunk 0 lands.
  - Tail: normalization split across Vector/Scalar halves with split
    output DMAs.
"""

import sys

if "/opt/trn_rl_repo" not in sys.path:
    sys.path.insert(0, "/opt/trn_rl_repo")

import ml_dtypes
import numpy as np

import concourse.bass as bass
import concourse.tile as tile
from concourse import mybir
from concourse.bass_utils import run_bass_kernel_spmd
from concourse.vector_clock import ScopedClock

# Problem shape (hardcoded per contract).
B, A, D, R = 16, 2048, 512, 10
NCORES = 8
PB = B // NCORES  # batches per core
P = 128
AT = A // P  # a-tiles (16)
CT = A // P  # c-tiles (16)
DC = D // P  # d-chunks (4)
SCALE = float(1.0 / np.sqrt(512.0))

F32 = mybir.dt.float32
DT = mybir.dt.bfloat16
NP_DT = ml_dtypes.bfloat16

WARMUP_MMS = 14


class PatchedTileContext(tile.TileContext):
    """Two fixes for this container's walrus build / perf:

    1. walrus rejects instructions carrying more than one semaphore
       sync-wait ("Too many sync wait commands"), and rejects ge-mode waits
       on InstDrain entirely. Hoist excess waits onto standalone
       EventSemaphore (wait) instructions emitted just before the owning
       instruction on the same engine.

    2. Tile splits every matmul into LDWEIGHTS+MATMUL and never dedups;
       walrus ldw-opt is disabled in this toolchain. Drop an LDWEIGHTS that
       reloads exactly the weights already in the PE array (sync-free ones
       only), so back-to-back matmuls sharing lhsT pay one weight load.
    """

    _wsplit_counter = 0

    def __init__(self, *args, **kwargs):
        super().__init__(*args, **kwargs)
        self._last_pe_weights = None
        self.n_ldw_dropped = 0

    def _split_excess_waits(self, inst, original_block):
        si = inst.sync_info
        if si is None:
            return
        waits = list(si.on_wait)
        if isinstance(inst, (mybir.InstDrain, mybir.InstNoOp)):
            keep = [w for w in waits if w.wait_mode == "sem-eq-imm"][:1]
        else:
            keep = waits[-1:]
        hoist = [w for w in waits if not any(w is k for k in keep)]
        if not hoist:
            return
        for w in hoist:
            PatchedTileContext._wsplit_counter += 1
            ev = mybir.InstEventSemaphore(
                name=f"I-wsplit-{PatchedTileContext._wsplit_counter}",
                engine=inst.engine,
            )
            ev.sync_info = mybir.SyncInfo(on_wait=[w], on_update=[])
            self.nc.register_instruction(ev)
            original_block.add_instruction(ev)
        inst.sync_info = mybir.SyncInfo(on_wait=keep, on_update=list(si.on_update))

    def _commit_and_lower(self, inst, original_block, old_bb_map, bb_to_exit_bb):
        if isinstance(inst, mybir.InstLdweights):
            si = inst.sync_info
            sync_free = si is None or (not si.on_wait and not si.on_update)
            key = str(inst.ins[0]) if inst.ins else None
            if (
                sync_free
                and key is not None
                and key == self._last_pe_weights
            ):
                self.n_ldw_dropped += 1
                return  # weights already resident in the PE array
            if key is not None and sync_free:
                self._last_pe_weights = key
            else:
                self._last_pe_weights = None
        elif isinstance(inst, mybir.InstMatmult):
            if getattr(inst, "is_transpose", False):
                # transpose-mode streams its input through the weight path
                self._last_pe_weights = None
        self._split_excess_waits(inst, original_block)
        return super()._commit_and_lower(inst, original_block, old_bb_map, bb_to_exit_bb)

    def _drain_and_barrier(self, tick_clock, wait_clock):
        probe = mybir.InstNoOp(name="I-tailprobe", engine=mybir.EngineType.SP)
        wait_clock.add_sem_waits(probe, ScopedClock({None: tick_clock.global_clock}))
        waits = probe.sync_info.on_wait if probe.sync_info else []
        allocated = self.sems.allocated()
        by_name = {}
        for key, h in allocated.items():
            by_name[str(key)] = h
            name = getattr(h, "name", None)
            if name is not None:
                by_name[str(name)] = h
        for w in waits:
            h = by_name.get(w.ant_name)
            assert h is not None, (w.ant_name, list(by_name)[:40])
            self.nc.sync.wait_ge(h, w.wait_value)
        self.nc.sync.drain()
        self.nc.all_engine_barrier()
        assert self.sems is not None
        popped = self.nc._tile_sem_poison_stack.pop()
        assert popped is self._sem_poison
        self.nc.clear_and_free_semaphores(list(allocated.values()))
        self.nc.all_engine_barrier()


def build_kernel() -> bass.Bass:
    nc = bass.Bass("TRN2", target_bir_lowering=False, debug=False)
    xs = nc.dram_tensor("xs", [PB, A, D], F32, kind="ExternalInput").ap()
    # wci rows 0:512 = wcat ([W1 | W2^T | 0] padded to 128 cols for FWL),
    # rows 512:640 = the 128x128 identity (transpose operand).
    wci = nc.dram_tensor("wci", [D + P, P], DT, kind="ExternalInput").ap()
    out = nc.dram_tensor("out", [PB, A, D], F32, kind="ExternalOutput").ap()

    Exp = mybir.ActivationFunctionType.Exp
    Copy = mybir.ActivationFunctionType.Copy

    with PatchedTileContext(nc) as tc:
        with (
            tc.tile_pool(name="consts", bufs=1) as consts,
            tc.tile_pool(name="xpool", bufs=1) as xpool,
            tc.tile_pool(name="xtpool", bufs=1) as xtpool,
            tc.tile_pool(name="lrpool", bufs=1) as lrpool,
            tc.tile_pool(name="ptpool", bufs=52) as ptpool,
            tc.tile_pool(name="smpool", bufs=4) as smpool,
            tc.tile_pool(name="outpool", bufs=3) as outpool,
            # one global PSUM pool; tags totalling 8 banks:
            #   st   [128,2,512] f32 x2 = 4 banks  (score ct-pairs)
            #   pv   [128, 512] f32 x2  = 2 banks  (PV out; warmup+proj reuse)
            #   sums [128,   1] f32 x2  = 2 banks  (PV sumexp; transposes reuse)
            tc.tile_pool(name="ps", bufs=2, space="PSUM") as ps,
        ):
            # ---- consts via one early DMA on the sync queue ----
            wcat_sb = consts.tile([P, DC, P], DT)
            ident = consts.tile([P, P], DT)
            nc.sync.dma_start(wcat_sb[:], wci[0:D].rearrange("(k p) m -> p k m", p=P))
            nc.sync.dma_start(ident[:], wci[D : D + P])
            junk = consts.tile([P, 512], DT)
            nc.vector.memset(junk[:], 0.0)
            ones_dt = consts.tile([P, 1], DT)
            nc.vector.memset(ones_dt[:], 1.0)
            # preload the scalar engine's activation table during the
            # DMA-bound head (the first real exp otherwise pays ~1.3us)
            preheat = smpool.tile([P, 1], F32, tag="recip", name="preheat")
            nc.scalar.activation(preheat[:], ones_dt[:], Exp, scale=1.0)

            # left/right score operands, allocated upfront so the zero fill
            # of right rows R.. can run early (kills the rightT garbage rows
            # of left during the K=128-padded score matmuls).
            lr = {}
            for b in range(PB):
                lr[b] = (
                    lrpool.tile([P, A], DT, name=f"left_{b}"),
                    lrpool.tile([P, A], DT, name=f"right_{b}"),
                )

            # ---- PE/HAM warm-up off the junk memset (ready ~1.5us before
            # the wcat DMA lands) ----
            wps = ps.tile([P, DC * P], F32, tag="pv", name="warm_ps")
            for _ in range(WARMUP_MMS):
                nc.tensor.matmul(wps[:], lhsT=junk[:, 0:P], rhs=junk[:], start=True, stop=True)

            # ---- x loads (cast f32 -> bf16 during DMA) ----
            # First two b0 tiles ride the sync queue (it boots ~3us before
            # the compute engines); the rest go through gpsimd DGE.
            x_tiles = []
            for b in range(PB):
                x_sb = xpool.tile([P, AT, D], DT, name=f"x_{b}")
                xr = xs[b].rearrange("(t p) d -> p t d", p=P)
                if b == 0:
                    chunks = [(0, 1), (1, 1), (2, 2), (4, 4), (8, 4), (12, 4)]
                else:
                    chunks = [(0, 2), (2, 2), (4, 2), (6, 2), (8, 4), (12, 4)]
                for lo, ln in chunks:
                    nc.gpsimd.dma_start(x_sb[:, lo : lo + ln, :], xr[:, lo : lo + ln, :])
                x_tiles.append(x_sb)
            # b1's right-zero fill on gpsimd after its x DMAs are queued
            nc.gpsimd.memset(lr[1][1][:], 0.0)

            xt_tiles = {}
            pts = {0: {}, 1: {}}

            # ---- step generators; emission order = per-engine program order ----

            def p1_steps(b):
                """4 groups of [4 transpose-tile steps, 1 projection-chunk step]."""

                def tr_step(t):
                    def go():
                        x_sb = x_tiles[b]
                        if t == 0:
                            xt_tiles[b] = xtpool.tile([P, DC, A], DT, tag="xt", name=f"xt_{b}")
                        tr = ps.tile([P, DC, P], DT, tag="sums", name=f"tr_{b}_{t}")
                        for dc in range(DC):
                            nc.tensor.transpose(
                                tr[:, dc, :], x_sb[:, t, dc * P : (dc + 1) * P], ident[:]
                            )
                        nc.vector.tensor_copy(xt_tiles[b][:, :, t * P : (t + 1) * P], tr[:])
                    return go

                def ms0():
                    # b0's right-zero fill on the vector queue, after the
                    # first 4 transpose copies (keeps them off the critical
                    # path to projection chunk 0).
                    nc.vector.memset(lr[0][1][:], 0.0)

                def pc_step(n4):
                    def go():
                        left_sb, right_sb = lr[b]
                        direct_right = b == 0 and n4 == 0
                        if direct_right:
                            # batch 0's first score matmul is on the critical
                            # path: produce right cols 0:512 straight from a
                            # second M=10 group instead of waiting for the
                            # row-shift DMA.
                            prd = ps.tile([R, 512], F32, tag="pv", name="prd_0")
                            for dc in range(DC):
                                nc.tensor.matmul(
                                    prd[:],
                                    lhsT=wcat_sb[:, dc, R : 2 * R],
                                    rhs=xt_tiles[b][:, dc, 0:512],
                                    start=(dc == 0),
                                    stop=(dc == DC - 1),
                                )
                            nc.scalar.copy(right_sb[0:R, 0:512], prd[:])
                        # M=128 projection chunk (cols 0-9 leftT, 10-19 right,
                        # 20-127 zeros -> left rows 20.. become exact zeros).
                        pchunk = ps.tile([P, 512], F32, tag="pv", name=f"prj_{b}_{n4}")
                        for dc in range(DC):
                            nc.tensor.matmul(
                                pchunk[:],
                                lhsT=wcat_sb[:, dc, :],
                                rhs=xt_tiles[b][:, dc, n4 * 512 : (n4 + 1) * 512],
                                start=(dc == 0),
                                stop=(dc == DC - 1),
                            )
                        sl = slice(n4 * 512, (n4 + 1) * 512)
                        nc.scalar.copy(left_sb[:, sl], pchunk[:])
                        # right rows (10-19) -> partitions 0-9 via SBUF->SBUF DMA
                        if not direct_right:
                            nc.sync.dma_start(right_sb[0:R, sl], left_sb[R : 2 * R, sl])
                    return go

                steps = []
                for n4 in range(4):
                    steps += [tr_step(4 * n4 + j) for j in range(4)]
                    if b == 0 and n4 == 0:
                        steps.append(ms0)
                    steps.append(pc_step(n4))
                return steps

            def st_steps(b):
                """(a-slice q, ct-pair cg) score steps, ordered by the x/proj
                chunk each needs (chunk n unlocks max(q, cg//2) == n)."""

                def st_step(q, cg):
                    def go():
                        left_sb, right_sb = lr[b]
                        st = ps.tile([P, 2, 512], F32, tag="st", name=f"st_{b}_{q}_{cg}")
                        for u in range(2):
                            ct = 2 * cg + u
                            nc.tensor.matmul(
                                st[:, u, :],
                                lhsT=right_sb[:, ct * P : (ct + 1) * P],
                                rhs=left_sb[:, q * 512 : (q + 1) * 512],
                                start=True,
                                stop=True,
                            )
                        pt = ptpool.tile([P, 2, 512], DT, tag="pt", name=f"pt_{b}_{q}_{cg}")
                        nc.scalar.activation(
                            pt[:].rearrange("p u n -> p (u n)"),
                            st[:].rearrange("p u n -> p (u n)"),
                            Exp,
                            scale=SCALE,
                        )
                        pts[b][(q, cg)] = pt
                    return go

                steps = []
                for n in range(4):
                    for q in range(4):
                        for cg in range(8):
                            if max(q, cg // 2) == n:
                                steps.append(st_step(q, cg))
                return steps

            def pv_steps(b):
                def pv_step(at):
                    def go():
                        x_sb = x_tiles[b]
                        q, j = at // 4, at % 4
                        ops = ps.tile([P, D], F32, tag="pv", name=f"ov_{b}_{at}")
                        sums = ps.tile([P, 1], F32, tag="sums", name=f"sm_{b}_{at}")
                        for ct in range(CT):
                            cg, u = ct // 2, ct % 2
                            w = pts[b][(q, cg)][:, u, j * P : (j + 1) * P]
                            # sums first: its reused-weights matmul is nearly
                            # free and lets the next LDWEIGHTS prefetch under
                            # the N=512 matmul.
                            nc.tensor.matmul(
                                sums[:], lhsT=w, rhs=ones_dt[:],
                                start=(ct == 0), stop=(ct == CT - 1),
                            )
                            nc.tensor.matmul(
                                ops[:], lhsT=w, rhs=x_sb[:, ct, :],
                                start=(ct == 0), stop=(ct == CT - 1),
                            )
                        recip = smpool.tile([P, 1], F32, tag="recip", name=f"rc_{b}_{at}")
                        nc.vector.reciprocal(recip[:], sums[:])
                        o_sb = outpool.tile([P, D], F32, tag="o", name=f"o_{b}_{at}")
                        half = D // 2
                        orow = out[b, at * P : (at + 1) * P, :]
                        nc.vector.tensor_scalar_mul(o_sb[:, 0:half], ops[:, 0:half], recip[:])
                        nc.sync.dma_start(orow[:, 0:half], o_sb[:, 0:half])
                        nc.scalar.activation(o_sb[:, half:D], ops[:, half:D], Copy, scale=recip[:])
                        nc.sync.dma_start(orow[:, half:D], o_sb[:, half:D])
                    return go

                return [pv_step(at) for at in range(AT)]

            A_ = p1_steps(0)   # 21 steps: 4x(4 tr + proj) + ms0
            C_ = p1_steps(1)   # 20
            D_ = pv_steps(0)   # 16
            F_ = pv_steps(1)   # 16
            # score steps grouped by a-slice q; each group of 8 is one
            # softmax-row's worth for 512 a's
            SBq = {}
            EQ = {}
            for q0 in range(4):
                SBq[q0] = []
                EQ[q0] = []
            for s, dst in ((st_steps(0), SBq), (st_steps(1), EQ)):
                i = 0
                for n in range(4):
                    for q0 in range(4):
                        for cg in range(8):
                            if max(q0, cg // 2) == n:
                                dst[q0].append(s[i])
                                i += 1

            # head: b0 transposes/projections with q=0 score steps threaded
            # in as their inputs land (chunk n unlocks 2 of them)
            for s in A_[0:6]:       # tr0-3, ms0, proj0
                s()
            SBq[0][0](); SBq[0][1]()
            for s in A_[6:11]:      # tr4-7, proj1
                s()
            SBq[0][2](); SBq[0][3]()
            for s in A_[11:16]:     # tr8-11, proj2
                s()
            SBq[0][4](); SBq[0][5]()
            for s in A_[16:21]:     # tr12-15, proj3
                s()
            SBq[0][6](); SBq[0][7]()
            # b0 PV starts immediately after its q0 scores (it needs nothing
            # from b1).  Each quarter of the PV phase carries the score
            # group needed 4 steps later (always >=1 full PV step of exp
            # slack), and b1's transposes/projections (whose x is still in
            # flight) thread into the first quarter instead of gating PV.
            dthread = SBq[1] + SBq[2] + SBq[3] + EQ[0]   # 32, 2 per D step
            cthread = list(C_)                           # 20, 5 into each of D_0..3
            for i, s in enumerate(D_):
                s()
                dthread[2 * i]()
                dthread[2 * i + 1]()
                for _ in range(5):
                    if cthread:
                        cthread.pop(0)()
            # b1 PV; remaining b1 scores thread into its first three quarters
            fthread = EQ[1] + EQ[2] + EQ[3]   # 24
            for i, s in enumerate(F_):
                s()
                if i < 12:
                    fthread[2 * i]()
                    fthread[2 * i + 1]()
    return nc


_NC_CACHE = None


def _get_nc():
    global _NC_CACHE
    if _NC_CACHE is None:
        _NC_CACHE = build_kernel()
    return _NC_CACHE


def make_in_maps(inputs):
    x = np.ascontiguousarray(np.asarray(inputs["x"], dtype=np.float32))
    W1 = np.asarray(inputs["W1"], dtype=np.float32)
    W2 = np.asarray(inputs["W2"], dtype=np.float32)
    wci = np.zeros((D + P, P), dtype=np.float32)
    wci[0:D, 0:R] = W1
    wci[0:D, R : 2 * R] = W2.T
    wci[D : D + P, 0:P] = np.eye(P, dtype=np.float32)
    wci = np.ascontiguousarray(wci.astype(NP_DT))
    return [
        {"xs": x[i * PB : (i + 1) * PB], "wci": wci} for i in range(NCORES)
    ]


def run(inputs, trace: bool = False):
    """Shard, execute on 8 cores, gather. Returns (out, BassKernelResults)."""
    nc = _get_nc()
    in_maps = make_in_maps(inputs)
    try:
        res = run_bass_kernel_spmd(nc, in_maps, core_ids=list(range(NCORES)), trace=trace)
    except Exception:
        # transient device hiccups (e.g. a wedged core from a prior run)
        # usually clear on retry
        res = run_bass_kernel_spmd(nc, in_maps, core_ids=list(range(NCORES)), trace=trace)
    full = np.concatenate([res.results[i]["out"] for i in range(NCORES)], axis=0)
    return full, res


def kernel(x, W1, W2):
    out, _ = run({"x": x, "W1": W1, "W2": W2})
    return out
